# revision 1
# baseline (speedup 1.0000x reference)
"""GNN unpool (gather by clique id + scatter-add by node id) on 8 trn2 cores.

Problem: inputs [B=16, C*NC], node_ids/clique_ids [M], output [B, N*C] where
  pooled = inputs.reshape(B, C, NC)
  out[b, c, node_ids[m]] += pooled[b, c, clique_ids[m]]  for each m

Sharding: 2 batch groups x 4 node ranges. Core (g, r) handles batches
[8g, 8g+8) (bc = 512 rows) and nodes [12544r, 12544(r+1)). This cuts the
per-core dma_gather index count 4x vs batch-only sharding: SWDGE descriptor
generation is a serial Q7 resource at ~7.8ns/index and is the pacing
engine; with 4x fewer + 4x larger (2KB) tokens it runs ~225us/core.

The host hands each core its batch-group's pooled features TRANSPOSED
(clique-major, [12544, 512] fp32) so the device needs no transpose phase at
all: dma_gather fetches 2KB fp32 token rows straight from the input, and
descriptor generation starts at t~0.

Device algorithm per core:
  1. dma_gather 2KB fp32 tokens for the core's membership entries grouped
     by node segment -> SBUF token tiles [128 entries, slot, 512 bc]
  2. entries are packed into a node-SEGMENT grid (2 blocks = 256 nodes per
     segment) whose per-segment chunk count is the max over the 4 node
     ranges -> identical compile-time structure on every core (SPMD), with
     per-core data (gather indices, one-hot offsets) in input tables.
     Segments are aligned descending-by-size per range to minimize padding.
     Per chunk: DVE builds one-hot H[entry, rel_node] fp32 via is_equal; PE
     matmuls H.T @ tokens (both bitcast float32r: full-rate rows at moving
     dim >= 256) accumulate psum [128 nodes, 512 bc] per block.
  3. ACT/DVE evacuate psum -> bf16 staging, DMA -> outT [12544, 512] bf16
     in segment-position order; host un-permutes rows, transposes, casts.
"""

import math
import sys

import numpy as np

sys.path.insert(0, "/opt/trn_rl_repo")

import ml_dtypes  # noqa: E402

from concourse import bacc, bass, mybir, tile  # noqa: E402
from concourse.bass_utils import run_bass_kernel_spmd  # noqa: E402

P = 128
N_CORES = 8
NGRP = 2  # batch groups
NRNG = 4  # node ranges
SEG_BLOCKS = 2  # node blocks per segment
SEG_W = SEG_BLOCKS * P  # 256 nodes per segment
GSZ = 8  # chunks per gather group (unused; calls are per position)


# ---------------------------------------------------------------- host planning


def _plan(node_ids, clique_ids, NC, N):
    node_ids = np.asarray(node_ids).astype(np.int64)
    clique_ids = np.asarray(clique_ids).astype(np.int64)
    M = node_ids.shape[0]

    NBLK_R = math.ceil(math.ceil(N / NRNG) / P)  # blocks per range (98)
    RW = NBLK_R * P  # nodes per range (12544)
    NSEG = math.ceil(NBLK_R / SEG_BLOCKS)  # segments per range (49)

    rng = node_ids // RW
    enode = node_ids - rng * RW
    seg = enode // SEG_W
    rel = enode - seg * SEG_W

    counts = np.zeros((NRNG, NSEG), np.int64)
    ent_clq = [[None] * NSEG for _ in range(NRNG)]
    ent_rel = [[None] * NSEG for _ in range(NRNG)]
    for r in range(NRNG):
        m_r = rng == r
        for s in range(NSEG):
            m_s = m_r & (seg == s)
            ent_clq[r][s] = clique_ids[m_s]
            ent_rel[r][s] = rel[m_s]
            counts[r, s] = int(m_s.sum())

    # Align segment positions descending by size per range: position p holds
    # each range's p-th largest segment, minimizing sum over p of max_r size.
    perm = np.argsort(-counts, axis=1, kind="stable")  # [NRNG, NSEG]
    sorted_counts = np.take_along_axis(counts, perm, axis=1)
    cap = np.max(sorted_counts, axis=0)  # [NSEG]
    nchunks = np.maximum(1, (cap + P - 1) // P)  # chunks per position

    seg_base = np.zeros(NSEG + 1, np.int64)  # first chunk of position p
    seg_base[1:] = np.cumsum(nchunks)
    CT = int(seg_base[NSEG])
    MPS = CT * P  # total gather slots

    idx_tbls = []
    nidrels = []
    for r in range(NRNG):
        stream = np.full(MPS, -1, np.int16)
        nid = np.full((P, CT), -2048.0, np.float32)
        for p in range(NSEG):
            s = int(perm[r, p])
            clqs = ent_clq[r][s].astype(np.int16)
            rels = ent_rel[r][s].astype(np.float32)
            n = len(clqs)
            base = int(seg_base[p]) * P
            # idx-0 pads up to the uniform reg count (cap, or the full
            # capacity for the first 8 positions so pool tiles are fully
            # initialized on first use); -1 beyond (truncated by the ucode,
            # consistent with num_idxs_reg)
            reg_n = int(nchunks[p]) * P if p < 8 else int(cap[p])
            stream[base : base + reg_n] = 0
            stream[base : base + n] = clqs
            padded = np.full(int(nchunks[p]) * P, -2048.0, np.float32)
            padded[:n] = rels
            nid[:, seg_base[p] : seg_base[p + 1]] = padded.reshape(-1, P).T
        wrapped = stream.reshape(-1, 16).T  # [16, MPS//16]
        idx_tbls.append(np.tile(wrapped, (8, 1)))  # [128, MPS//16]
        nidrels.append(nid)

    iota = np.tile(np.arange(SEG_W, dtype=np.float32)[None, :], (P, 1))

    # one gather call per segment position; num_idxs = num_idxs_reg =
    # the uniform real count (cap) so the decode ring reservation matches
    # the ucode descriptor count on every core and no pad descriptors are
    # generated. First 8 positions fetch fully to initialize the 8 pool
    # tiles (avoids uninitialized-SBUF operands).
    groups = []
    regs = []
    for p in range(NSEG):
        groups.append((int(seg_base[p]), int(seg_base[p + 1])))
        regs.append(
            int(nchunks[p]) * P if p < 8 else int(cap[p])
        )

    return dict(
        M=M,
        NC=NC,
        N=N,
        NBLK_R=NBLK_R,
        RW=RW,
        NSEG=NSEG,
        perm=perm,
        nchunks=nchunks,
        seg_base=seg_base,
        CT=CT,
        MPS=MPS,
        idx_tbls=idx_tbls,
        nidrels=nidrels,
        iota=iota,
        groups=groups,
        regs=regs,
    )


# ---------------------------------------------------------------- device build


def _build(plan):
    NBLK_R = plan["NBLK_R"]
    NSEG = plan["NSEG"]
    nchunks = plan["nchunks"]
    seg_base = plan["seg_base"]
    CT = plan["CT"]
    MPS = plan["MPS"]
    groups = plan["groups"]
    regs = plan["regs"]

    BC = 4 * P  # 512 bc rows per core
    NCP = plan["RW"]  # poolT rows = padded clique count? no: clique rows

    f32 = mybir.dt.float32
    f32r = mybir.dt.float32r
    bf16 = mybir.dt.bfloat16
    i16 = mybir.dt.int16

    NCROWS = math.ceil(plan["NC"] / P) * P  # 12544 padded clique rows

    nc = bacc.Bacc(None, target_bir_lowering=False)

    poolT_d = nc.dram_tensor("pooledT", [NCROWS, BC], f32, kind="ExternalInput")
    idx_d = nc.dram_tensor("idxtbl", [P, MPS // 16], i16, kind="ExternalInput")
    widx_d = nc.dram_tensor("warmidx", [P, 8], i16, kind="ExternalInput")
    nidrel_d = nc.dram_tensor("nidrel", [P, CT], f32, kind="ExternalInput")
    iota_d = nc.dram_tensor("iotatbl", [P, SEG_W], f32, kind="ExternalInput")
    out_d = nc.dram_tensor("out", [NBLK_R * P, BC], bf16, kind="ExternalOutput")

    with tile.TileContext(nc) as tc:
        with (
            tc.tile_pool(name="const", bufs=1) as constp,
            tc.tile_pool(name="upool", bufs=8) as upool,
            tc.tile_pool(name="hpool", bufs=8) as hpool,
            tc.tile_pool(name="opsum", bufs=8, space="PSUM") as opsum,
            tc.tile_pool(name="stage", bufs=3) as stagep,
        ):
            widx_t = constp.tile([P, 8], i16)
            nc.sync.dma_start(widx_t[:], widx_d[:])
            wut = constp.tile([P, 1, BC], f32r)
            nc.gpsimd.dma_gather(
                out_ap=wut[:, :, :],
                in_ap=poolT_d[:].bitcast(f32r),
                idxs_ap=widx_t[:],
                num_idxs=P,
                num_idxs_reg=P,
                elem_size=BC,
                single_packet=False,
            )
            idx_t = constp.tile([P, MPS // 16], i16)
            nc.sync.dma_start(idx_t[:], idx_d[:])
            iota_t = constp.tile([P, SEG_W], f32)
            nc.sync.dma_start(iota_t[:], iota_d[:])
            nidrel_t = constp.tile([P, CT], f32)
            nc.sync.dma_start(nidrel_t[:], nidrel_d[:])

            # ---- gathers: 2KB fp32 tokens straight from the input ----
            NCKMAX = int(max(nchunks))
            u_tiles = []
            for gi, (c0, c1) in enumerate(groups):
                nst = c1 - c0
                ut = upool.tile([P, NCKMAX, BC], f32r, tag="utok")
                nc.gpsimd.dma_gather(
                    out_ap=ut[:, :nst, :],
                    in_ap=poolT_d[:].bitcast(f32r),
                    idxs_ap=idx_t[:, c0 * 8 : c1 * 8],
                    num_idxs=regs[gi],
                    num_idxs_reg=regs[gi],
                    elem_size=BC,
                    single_packet=False,
                )
                u_tiles.append(ut)

            # ---- one-hot matmul scatter per segment position ----
            SGRP = 8  # blocks per output staging tile
            cur_stage = None
            cur_blk0 = 0
            blk = 0
            for p in range(NSEG):
                nck = int(nchunks[p])
                pq = [
                    opsum.tile([P, BC], f32, tag="ops", name=f"pq{p}_{b}")
                    for b in range(SEG_BLOCKS)
                ]
                for local in range(nck):
                    c = int(seg_base[p]) + local
                    gi = p
                    sl = local
                    ht = hpool.tile([P, SEG_W], f32r, tag="h")
                    nc.vector.tensor_scalar(
                        out=ht[:],
                        in0=iota_t[:],
                        scalar1=nidrel_t[:, c : c + 1],
                        scalar2=None,
                        op0=mybir.AluOpType.is_equal,
                    )
                    ut = u_tiles[gi]
                    for b in range(SEG_BLOCKS):
                        nc.tensor.matmul(
                            out=pq[b][:],
                            lhsT=ht[:, b * P : (b + 1) * P],
                            rhs=ut[:, sl, :],
                            start=(local == 0),
                            stop=(local == nck - 1),
                        )
                for b in range(SEG_BLOCKS):
                    if cur_stage is None:
                        cur_stage = stagep.tile([P, SGRP, BC], bf16, tag="st")
                        cur_blk0 = blk
                    if blk % 2 == 0:
                        nc.scalar.copy(cur_stage[:, blk - cur_blk0, :], pq[b][:])
                    else:
                        nc.vector.tensor_copy(
                            cur_stage[:, blk - cur_blk0, :], pq[b][:]
                        )
                    blk += 1
                    if blk - cur_blk0 == SGRP or blk == NBLK_R:
                        nb = blk - cur_blk0
                        nc.sync.dma_start(
                            out_d[cur_blk0 * P : blk * P, :].rearrange(
                                "(t r) c -> r t c", t=nb
                            ),
                            cur_stage[:, :nb, :],
                        )
                        cur_stage = None

    nc.finalize()
    return nc


# ---------------------------------------------------------------- entry points

_CACHE = {}


def _get_program(inputs):
    inputs_arr = np.asarray(inputs["inputs"])
    node_ids = np.asarray(inputs["node_ids"])
    clique_ids = np.asarray(inputs["clique_ids"])
    N = int(inputs["nodes"])
    C = int(inputs["n_channels"])
    B, units_dim = inputs_arr.shape
    NC = units_dim // C

    key = (
        B,
        C,
        NC,
        N,
        node_ids.shape[0],
        hash(node_ids.tobytes()),
        hash(clique_ids.tobytes()),
    )
    if key not in _CACHE:
        plan = _plan(node_ids, clique_ids, NC, N)
        nc = _build(plan)
        _CACHE[key] = (plan, nc)
    return _CACHE[key]


def _run(inputs, trace=False):
    inputs_arr = np.asarray(inputs["inputs"]).astype(np.float32)
    N = int(inputs["nodes"])
    C = int(inputs["n_channels"])
    B = inputs_arr.shape[0]
    NC = inputs_arr.shape[1] // C
    b_grp = B // NGRP  # batches per group (8)

    plan, nc = _get_program(inputs)
    RW = plan["RW"]
    NSEG = plan["NSEG"]
    perm = plan["perm"]
    NCROWS = math.ceil(NC / P) * P

    # host-side sharding layout: per batch group, clique-major fp32
    poolTs = []
    for g in range(NGRP):
        pooled = inputs_arr[g * b_grp : (g + 1) * b_grp].reshape(b_grp * C, NC)
        pt = np.zeros((NCROWS, b_grp * C), np.float32)
        pt[:NC] = pooled.T
        poolTs.append(pt)

    in_maps = []
    for d in range(N_CORES):
        g, r = d // NRNG, d % NRNG
        in_maps.append(
            {
                "pooledT": poolTs[g],
                "idxtbl": plan["idx_tbls"][r],
                "warmidx": np.ascontiguousarray(plan["idx_tbls"][r][:, :8]),
                "nidrel": plan["nidrels"][r],
                "iotatbl": plan["iota"],
            }
        )

    res = run_bass_kernel_spmd(
        nc, in_maps, core_ids=list(range(N_CORES)), trace=trace
    )

    out = np.empty((B, C, N), np.float32)
    for d in range(N_CORES):
        g, r = d // NRNG, d % NRNG
        o = np.asarray(res.results[d]["out"]).astype(np.float32)
        # outT rows [SEG_W*p : SEG_W*(p+1)] hold real segment perm[r][p]
        osegs = o.reshape(NSEG, SEG_W, b_grp * C)
        unperm = np.empty_like(osegs)
        unperm[perm[r]] = osegs
        full = unperm.reshape(NSEG * SEG_W, b_grp * C)  # [12544, 512]
        w = min(RW, N - r * RW)
        out[g * b_grp : (g + 1) * b_grp, :, r * RW : r * RW + w] = (
            full[:w].T.reshape(b_grp, C, w)
        )
    return out.reshape(B, C * N), res


def kernel(**inputs) -> np.ndarray:
    out, _ = _run(inputs, trace=False)
    return out



# revision 4
# speedup vs baseline: 1.5438x; 1.5438x over previous
"""GNN unpool (gather by clique id + scatter-add by node id) on 8 trn2 cores.

Problem: inputs [B=16, C*NC], node_ids/clique_ids [M], output [B, N*C] where
  pooled = inputs.reshape(B, C, NC)
  out[b, c, node_ids[m]] += pooled[b, c, clique_ids[m]]  for each m

Sharding: 8 node ranges x full batch. Core r handles nodes
[6250r, 6250(r+1)) for all 16 batches (bc = 1024 rows). Per-core membership
entries ~= M/8 = 12500, which halves the SWDGE dma_gather index count vs the
old 2x4 sharding: Q7 descriptor generation (~7.5ns/index + ~0.7us/call) was
the pacing engine at 287us.

The host hands every core the same clique-major pooled features in bf16
([NC, 1024]); dma_gather fetches 2KB bf16 token rows per entry.

Entries are packed per core into variable-width node segments (<=128 nodes,
greedy-filled to <=256 entries) so every segment is exactly 2 chunks of 128
gather slots -- no chunk-grid padding (a fixed 128-node segment averages
256 +- 16 entries, straddling the 2-chunk boundary and wasting ~25% slots).
Segment positions are uniform across cores (SPMD): chunk grid is the max
over cores; short cores pad with idx-0 gathers and all-zero one-hot columns.

Scatter is a one-hot matmul: host PRE-COMPUTES the one-hot H[entry, rel]
tables (bf16) and DMAs them in (the old kernel built them on DVE via
is_equal at ~1us each -- 215us of DVE busy, the #2 bottleneck). Per chunk:
PE matmuls H_c.T [128e x 128n] @ tok_c [128e, 512|512] into two psum banks
(bf16 in, fp32 accum). ACT+DVE evacuate psum -> bf16 staging, DMA -> out
[NSEG*128, 1024] bf16 in segment-position order; host drops pad rows,
transposes, casts.
"""

import math
import sys

import numpy as np

sys.path.insert(0, "/opt/trn_rl_repo")

import ml_dtypes  # noqa: E402

from concourse import bacc, bass, mybir, tile  # noqa: E402
from concourse.bass_utils import run_bass_kernel_spmd  # noqa: E402

P = 128
N_CORES = 8
BC = 1024  # full batch x channels = 16*64
SEG_CAP = 2 * P  # max entries per segment (2 gather chunks)
GPOS = 8  # segment positions per gather call
SGRP = 8  # positions per output staging tile


# ---------------------------------------------------------------- host planning


def _plan(node_ids, clique_ids, NC, N):
    node_ids = np.asarray(node_ids).astype(np.int64)
    clique_ids = np.asarray(clique_ids).astype(np.int64)
    M = node_ids.shape[0]
    NR = N // N_CORES  # nodes per core (6250)
    assert NR * N_CORES == N

    rng = node_ids // NR
    enode = node_ids - rng * NR

    # Per-core greedy segmentation: contiguous node windows, <=128 nodes,
    # <=SEG_CAP entries each.
    per_core = []
    nseg_r = []
    for r in range(N_CORES):
        m = rng == r
        en = enode[m]
        cl = clique_ids[m]
        order = np.argsort(en, kind="stable")
        en = en[order]
        cl = cl[order]
        cnt = np.bincount(en, minlength=NR)
        cum = np.concatenate([[0], np.cumsum(cnt)])  # entries before node i
        starts = []
        widths = []
        s = 0
        while s < NR:
            e = int(np.searchsorted(cum, cum[s] + SEG_CAP, side="right")) - 1
            e = min(max(e, s + 1), s + P, NR)
            starts.append(s)
            widths.append(e - s)
            s = e
        starts = np.array(starts, np.int64)
        widths = np.array(widths, np.int64)
        scount = cum[starts + widths] - cum[starts]
        assert scount.max() <= SEG_CAP
        per_core.append(dict(en=en, cl=cl, cum=cum, starts=starts,
                             widths=widths, scount=scount))
        nseg_r.append(len(starts))

    NSEG = max(nseg_r)
    counts = np.zeros((N_CORES, NSEG), np.int64)
    for r in range(N_CORES):
        counts[r, : nseg_r[r]] = per_core[r]["scount"]
    cap = counts.max(axis=0)
    nchunks = np.maximum(1, (cap + P - 1) // P)  # expected: all 2
    seg_base = np.zeros(NSEG + 1, np.int64)
    seg_base[1:] = np.cumsum(nchunks)
    CT = int(seg_base[NSEG])

    idx_tbls = []
    h_tbls = []
    for r in range(N_CORES):
        pc = per_core[r]
        en, cl, cum = pc["en"], pc["cl"], pc["cum"]
        starts, widths = pc["starts"], pc["widths"]
        nsr = nseg_r[r]
        # entry j (node-sorted) -> segment p, slot seg_base[p]*128 + rank
        p_of = np.searchsorted(starts + widths, en, side="right")
        ebase = cum[starts]  # first entry index of each segment
        j = np.arange(len(en))
        slots = seg_base[p_of] * P + (j - ebase[p_of])
        stream = np.zeros(CT * P, np.int16)
        stream[slots] = cl.astype(np.int16)
        wrapped = stream.reshape(-1, 16).T  # [16, CT*8]
        idx_tbls.append(np.tile(wrapped, (8, 1)).copy())  # [128, CT*8]

        h_flat = np.zeros((CT * P, P), np.float32)
        h_flat[slots, en - starts[p_of]] = 1.0
        h_tile = (
            h_flat.reshape(CT, P, P).transpose(1, 0, 2).reshape(P, CT * P)
        )
        h_tbls.append(h_tile.astype(ml_dtypes.bfloat16))

    # Small leading groups shrink the pipeline-fill bubble (PE can start
    # after a 2-position gather instead of an 8-position one); a small tail
    # group shrinks the drain.
    groups = []
    p0 = 0
    for sz in (2, 2, 4):
        if p0 >= NSEG:
            break
        p1 = min(p0 + sz, NSEG)
        groups.append((p0, p1))
        p0 = p1
    while p0 < NSEG:
        p1 = min(p0 + GPOS, NSEG)
        if NSEG - p0 > GPOS and NSEG - p0 < GPOS + 4:
            p1 = p0 + (NSEG - p0 + 1) // 2  # split remainder evenly
        groups.append((p0, p1))
        p0 = p1

    return dict(
        M=M,
        NC=NC,
        N=N,
        NR=NR,
        NSEG=NSEG,
        nseg_r=nseg_r,
        per_core=per_core,
        nchunks=nchunks,
        seg_base=seg_base,
        CT=CT,
        idx_tbls=idx_tbls,
        h_tbls=h_tbls,
        groups=groups,
    )


# ---------------------------------------------------------------- device build


def _build(plan):
    NSEG = plan["NSEG"]
    seg_base = plan["seg_base"]
    CT = plan["CT"]
    groups = plan["groups"]

    f32 = mybir.dt.float32
    bf16 = mybir.dt.bfloat16
    i16 = mybir.dt.int16

    NCKG = max(int(seg_base[p1] - seg_base[p0]) for p0, p1 in groups)

    nc = bacc.Bacc(None, target_bir_lowering=False)

    poolT_d = nc.dram_tensor("pooledT", [plan["NC"], BC], bf16,
                             kind="ExternalInput")
    idx_d = nc.dram_tensor("idxtbl", [P, CT * 8], i16, kind="ExternalInput")
    h_d = nc.dram_tensor("htbl", [P, CT * P], bf16, kind="ExternalInput")
    out_d = nc.dram_tensor("out", [NSEG * P, BC], bf16, kind="ExternalOutput")

    with tile.TileContext(nc) as tc:
        with (
            tc.tile_pool(name="const", bufs=1) as constp,
            tc.tile_pool(name="tok", bufs=3) as tokp,
            tc.tile_pool(name="hp", bufs=3) as hp,
            tc.tile_pool(name="opsum", bufs=8, space="PSUM") as opsum,
            tc.tile_pool(name="stage", bufs=2) as stagep,
        ):
            idx_t = constp.tile([P, CT * 8], i16)
            nc.sync.dma_start(idx_t[:], idx_d[:])

            ngrp = len(groups)

            # Issue all gathers up-front on the gpsimd queue: Q7 descriptor
            # generation is the serial pacer and must never idle. Pool-buffer
            # reuse (bufs=3) throttles them against PE consumption.
            tok_tiles = []
            for p0, p1 in groups:
                c0, c1 = int(seg_base[p0]), int(seg_base[p1])
                nck = c1 - c0
                ut = tokp.tile([P, NCKG, BC], bf16, tag="tok")
                nc.gpsimd.dma_gather(
                    out_ap=ut[:, :nck, :],
                    in_ap=poolT_d[:],
                    idxs_ap=idx_t[:, c0 * 8 : c1 * 8],
                    num_idxs=nck * P,
                    num_idxs_reg=nck * P,
                    elem_size=BC,
                    single_packet=False,
                )
                tok_tiles.append(ut)

            # One-hot tables ride the sync queue, prefetched 2 groups deep so
            # the sync-FIFO order (H loads interleaved with output writes)
            # matches dependency resolution order (no FIFO deadlock).
            h_tiles = []

            def _load_h(gi):
                p0, p1 = groups[gi]
                c0, c1 = int(seg_base[p0]), int(seg_base[p1])
                nck = c1 - c0
                ht = hp.tile([P, NCKG * P], bf16, tag="h")
                nc.sync.dma_start(ht[:, : nck * P], h_d[:, c0 * P : c1 * P])
                h_tiles.append(ht)

            _load_h(0)
            if ngrp > 1:
                _load_h(1)

            cur_stage = None
            p_st = 0
            for gi, (p0, p1) in enumerate(groups):
                if gi + 2 < ngrp:
                    _load_h(gi + 2)
                c0 = int(seg_base[p0])
                ut = tok_tiles[gi]
                ht = h_tiles[gi]
                for p in range(p0, p1):
                    ca, cb = int(seg_base[p]), int(seg_base[p + 1])
                    pq0 = opsum.tile([P, BC // 2], f32, tag="ops")
                    pq1 = opsum.tile([P, BC // 2], f32, tag="ops")
                    for c in range(ca, cb):
                        lc = c - c0
                        hs = ht[:, lc * P : (lc + 1) * P]
                        nc.tensor.matmul(
                            out=pq0[:],
                            lhsT=hs,
                            rhs=ut[:, lc, : BC // 2],
                            start=(c == ca),
                            stop=(c == cb - 1),
                        )
                        nc.tensor.matmul(
                            out=pq1[:],
                            lhsT=hs,
                            rhs=ut[:, lc, BC // 2 :],
                            start=(c == ca),
                            stop=(c == cb - 1),
                        )
                    if cur_stage is None:
                        cur_stage = stagep.tile([P, SGRP, BC], bf16, tag="st")
                        p_st = p
                    nc.scalar.copy(cur_stage[:, p - p_st, : BC // 2], pq0[:])
                    nc.vector.tensor_copy(
                        cur_stage[:, p - p_st, BC // 2 :], pq1[:]
                    )
                    if p - p_st + 1 == SGRP or p == NSEG - 1:
                        nb = p - p_st + 1
                        nc.sync.dma_start(
                            out_d[p_st * P : (p + 1) * P, :].rearrange(
                                "(t r) c -> r t c", t=nb
                            ),
                            cur_stage[:, :nb, :],
                        )
                        cur_stage = None

    nc.finalize()
    return nc


# ---------------------------------------------------------------- entry points

_CACHE = {}


def _get_program(inputs):
    inputs_arr = np.asarray(inputs["inputs"])
    node_ids = np.asarray(inputs["node_ids"])
    clique_ids = np.asarray(inputs["clique_ids"])
    N = int(inputs["nodes"])
    C = int(inputs["n_channels"])
    B, units_dim = inputs_arr.shape
    NC = units_dim // C

    key = (
        B,
        C,
        NC,
        N,
        node_ids.shape[0],
        hash(node_ids.tobytes()),
        hash(clique_ids.tobytes()),
    )
    if key not in _CACHE:
        plan = _plan(node_ids, clique_ids, NC, N)
        nc = _build(plan)
        _CACHE[key] = (plan, nc)
    return _CACHE[key]


def _run(inputs, trace=False):
    inputs_arr = np.asarray(inputs["inputs"]).astype(np.float32)
    N = int(inputs["nodes"])
    C = int(inputs["n_channels"])
    B = inputs_arr.shape[0]
    NC = inputs_arr.shape[1] // C

    plan, nc = _get_program(inputs)
    NR = plan["NR"]
    nseg_r = plan["nseg_r"]

    # clique-major bf16 pooled features, shared by every core
    poolT = np.ascontiguousarray(
        inputs_arr.reshape(B * C, NC).T
    ).astype(ml_dtypes.bfloat16)

    in_maps = []
    for r in range(N_CORES):
        in_maps.append(
            {
                "pooledT": poolT,
                "idxtbl": plan["idx_tbls"][r],
                "htbl": plan["h_tbls"][r],
            }
        )

    res = run_bass_kernel_spmd(
        nc, in_maps, core_ids=list(range(N_CORES)), trace=trace
    )

    out = np.empty((B, C, N), np.float32)
    for r in range(N_CORES):
        pc = plan["per_core"][r]
        starts, widths = pc["starts"], pc["widths"]
        o = np.asarray(res.results[r]["out"]).astype(np.float32)
        blocks = o.reshape(plan["NSEG"], P, B * C)
        full = np.concatenate(
            [blocks[p, : widths[p]] for p in range(nseg_r[r])], axis=0
        )  # [NR, 1024] in node order
        out[:, :, r * NR : (r + 1) * NR] = full.T.reshape(B, C, NR)
    return out.reshape(B, C * N), res


def kernel(**inputs) -> np.ndarray:
    out, _ = _run(inputs, trace=False)
    return out


# revision 6
# speedup vs baseline: 1.7673x; 1.1448x over previous
"""GNN unpool (gather by clique id + scatter-add by node id) on 8 trn2 cores.

Problem: inputs [B=16, C*NC], node_ids/clique_ids [M], output [B, N*C] where
  pooled = inputs.reshape(B, C, NC)
  out[b, c, node_ids[m]] += pooled[b, c, clique_ids[m]]  for each m

Sharding: 8 node ranges x full batch. Core r handles nodes
[6250r, 6250(r+1)) for all 16 batches (bc = 1024 rows). Per-core membership
entries ~= M/8 = 12500, which halves the SWDGE dma_gather index count vs the
old 2x4 sharding: Q7 descriptor generation (~7.5ns/index + ~0.7us/call) was
the pacing engine at 287us.

The host hands every core the same clique-major pooled features in bf16
([NC, 1024]); dma_gather fetches 2KB bf16 token rows per entry.

Entries are packed per core into variable-width node segments (<=128 nodes,
greedy-filled to <=256 entries) so every segment is exactly 2 chunks of 128
gather slots -- no chunk-grid padding (a fixed 128-node segment averages
256 +- 16 entries, straddling the 2-chunk boundary and wasting ~25% slots).
Segment positions are uniform across cores (SPMD): chunk grid is the max
over cores; short cores pad with idx-0 gathers and all-zero one-hot columns.

Scatter is a one-hot matmul: host PRE-COMPUTES the one-hot H[entry, rel]
tables (bf16) and DMAs them in (the old kernel built them on DVE via
is_equal at ~1us each -- 215us of DVE busy, the #2 bottleneck). Per chunk:
PE matmuls H_c.T [128e x 128n] @ tok_c [128e, 512|512] into two psum banks
(bf16 in, fp32 accum). ACT+DVE evacuate psum -> bf16 staging, DMA -> out
[NSEG*128, 1024] bf16 in segment-position order; host drops pad rows,
transposes, casts.
"""

import math
import sys

import numpy as np

sys.path.insert(0, "/opt/trn_rl_repo")

import ml_dtypes  # noqa: E402

from concourse import bacc, bass, mybir, tile  # noqa: E402
from concourse.bass_utils import run_bass_kernel_spmd  # noqa: E402

P = 128
N_CORES = 8
BC = 1024  # full batch x channels = 16*64
SEG_CAP = 2 * P  # max entries per segment (2 gather chunks)
GPOS = 8  # segment positions per gather call
SGRP = 8  # positions per output staging tile


# ---------------------------------------------------------------- host planning


def _plan(node_ids, clique_ids, NC, N):
    node_ids = np.asarray(node_ids).astype(np.int64)
    clique_ids = np.asarray(clique_ids).astype(np.int64)
    M = node_ids.shape[0]
    NR = N // N_CORES  # nodes per core (6250)
    assert NR * N_CORES == N

    rng = node_ids // NR
    enode = node_ids - rng * NR

    # Per-core greedy segmentation: contiguous node windows, <=128 nodes,
    # <=SEG_CAP entries each.
    per_core = []
    nseg_r = []
    for r in range(N_CORES):
        m = rng == r
        en = enode[m]
        cl = clique_ids[m]
        order = np.argsort(en, kind="stable")
        en = en[order]
        cl = cl[order]
        cnt = np.bincount(en, minlength=NR)
        cum = np.concatenate([[0], np.cumsum(cnt)])  # entries before node i
        starts = []
        widths = []
        s = 0
        while s < NR:
            e = int(np.searchsorted(cum, cum[s] + SEG_CAP, side="right")) - 1
            e = min(max(e, s + 1), s + P, NR)
            starts.append(s)
            widths.append(e - s)
            s = e
        starts = np.array(starts, np.int64)
        widths = np.array(widths, np.int64)
        scount = cum[starts + widths] - cum[starts]
        assert scount.max() <= SEG_CAP
        per_core.append(dict(en=en, cl=cl, cum=cum, starts=starts,
                             widths=widths, scount=scount))
        nseg_r.append(len(starts))

    NSEG = max(nseg_r)
    counts = np.zeros((N_CORES, NSEG), np.int64)
    for r in range(N_CORES):
        counts[r, : nseg_r[r]] = per_core[r]["scount"]
    cap = counts.max(axis=0)
    nchunks = np.maximum(1, (cap + P - 1) // P)  # expected: all 2
    seg_base = np.zeros(NSEG + 1, np.int64)
    seg_base[1:] = np.cumsum(nchunks)
    CT = int(seg_base[NSEG])

    idx_tbls = []
    h_tbls = []
    for r in range(N_CORES):
        pc = per_core[r]
        en, cl, cum = pc["en"], pc["cl"], pc["cum"]
        starts, widths = pc["starts"], pc["widths"]
        nsr = nseg_r[r]
        # entry j (node-sorted) -> segment p, slot seg_base[p]*128 + rank
        p_of = np.searchsorted(starts + widths, en, side="right")
        ebase = cum[starts]  # first entry index of each segment
        j = np.arange(len(en))
        slots = seg_base[p_of] * P + (j - ebase[p_of])
        stream = np.zeros(CT * P, np.int16)
        stream[slots] = cl.astype(np.int16)
        wrapped = stream.reshape(-1, 16).T  # [16, CT*8]
        idx_tbls.append(np.tile(wrapped, (8, 1)).copy())  # [128, CT*8]

        h_flat = np.zeros((CT * P, P), np.float32)
        h_flat[slots, en - starts[p_of]] = 1.0
        h_tile = (
            h_flat.reshape(CT, P, P).transpose(1, 0, 2).reshape(P, CT * P)
        )
        h_tbls.append(h_tile.astype(ml_dtypes.bfloat16))

    # Small leading groups shrink the pipeline-fill bubble (PE can start
    # after a 2-position gather instead of an 8-position one); tapered tail
    # groups shrink the drain (last gather's transfer+PE+evac+write chain).
    front = [2, 2, 4]
    tail = [4, 3, 2]
    mid = NSEG - sum(front) - sum(tail)
    if mid >= 0:
        sizes = front + [GPOS] * (mid // GPOS)
        if mid % GPOS:
            sizes.append(mid % GPOS)
        sizes += tail
    else:
        sizes = []
        rem = NSEG
        while rem > 0:
            sizes.append(min(4, rem))
            rem -= sizes[-1]
    groups = []
    p0 = 0
    for sz in sizes:
        groups.append((p0, p0 + sz))
        p0 += sz
    assert p0 == NSEG

    return dict(
        M=M,
        NC=NC,
        N=N,
        NR=NR,
        NSEG=NSEG,
        nseg_r=nseg_r,
        per_core=per_core,
        nchunks=nchunks,
        seg_base=seg_base,
        CT=CT,
        idx_tbls=idx_tbls,
        h_tbls=h_tbls,
        groups=groups,
    )


# ---------------------------------------------------------------- device build


def _build(plan):
    NSEG = plan["NSEG"]
    seg_base = plan["seg_base"]
    CT = plan["CT"]
    groups = plan["groups"]

    f32 = mybir.dt.float32
    bf16 = mybir.dt.bfloat16
    i16 = mybir.dt.int16

    NCKG = max(int(seg_base[p1] - seg_base[p0]) for p0, p1 in groups)

    nc = bacc.Bacc(None, target_bir_lowering=False)

    poolT_d = nc.dram_tensor("pooledT", [plan["NC"], BC], bf16,
                             kind="ExternalInput")
    idx_d = nc.dram_tensor("idxtbl", [P, CT * 8], i16, kind="ExternalInput")
    h_d = nc.dram_tensor("htbl", [P, CT * P], bf16, kind="ExternalInput")
    out_d = nc.dram_tensor("out", [NSEG * P, BC], bf16, kind="ExternalOutput")

    with tile.TileContext(nc) as tc:
        with (
            tc.tile_pool(name="const", bufs=1) as constp,
            tc.tile_pool(name="tok", bufs=3) as tokp,
            tc.tile_pool(name="hp", bufs=3) as hp,
            tc.tile_pool(name="opsum", bufs=8, space="PSUM") as opsum,
            tc.tile_pool(name="stage", bufs=2) as stagep,
        ):
            f32r = mybir.dt.float32r

            idx_t = constp.tile([P, CT * 8], i16)
            nc.sync.dma_start(idx_t[:], idx_d[:])

            # Warm-up gather with an on-chip zero index tile: absorbs the
            # ~6us gather-ucode IRAM load while the real idx table is still
            # in flight. All-zero idxs fetch row 0 into a scratch tile.
            widx = constp.tile([P, 8], i16)
            nc.vector.memset(widx[:], 0)
            wtok = constp.tile([P, 1, BC], bf16)
            nc.gpsimd.dma_gather(
                out_ap=wtok[:, :, :].bitcast(f32r),
                in_ap=poolT_d[:].bitcast(f32r),
                idxs_ap=widx[:],
                num_idxs=P,
                num_idxs_reg=P,
                elem_size=BC // 2,
                single_packet=False,
            )

            ngrp = len(groups)

            # Issue all gathers up-front on the gpsimd queue: Q7 descriptor
            # generation is the serial pacer and must never idle. Pool-buffer
            # reuse (bufs=3) throttles them against PE consumption. The
            # f32r bitcast (2048B rows as 512x4B) keeps the ucode on the
            # 4-byte path: the 2-byte path generates descriptors ~25% slower
            # (9.3 vs 7.5 ns/idx measured).
            tok_tiles = []
            for p0, p1 in groups:
                c0, c1 = int(seg_base[p0]), int(seg_base[p1])
                nck = c1 - c0
                ut = tokp.tile([P, NCKG, BC], bf16, tag="tok")
                nc.gpsimd.dma_gather(
                    out_ap=ut[:, :nck, :].bitcast(f32r),
                    in_ap=poolT_d[:].bitcast(f32r),
                    idxs_ap=idx_t[:, c0 * 8 : c1 * 8],
                    num_idxs=nck * P,
                    num_idxs_reg=nck * P,
                    elem_size=BC // 2,
                    single_packet=False,
                )
                tok_tiles.append(ut)

            # One-hot tables ride the sync queue, prefetched 2 groups deep so
            # the sync-FIFO order (H loads interleaved with output writes)
            # matches dependency resolution order (no FIFO deadlock).
            h_tiles = []

            def _load_h(gi):
                p0, p1 = groups[gi]
                c0, c1 = int(seg_base[p0]), int(seg_base[p1])
                nck = c1 - c0
                ht = hp.tile([P, NCKG * P], bf16, tag="h")
                nc.sync.dma_start(ht[:, : nck * P], h_d[:, c0 * P : c1 * P])
                h_tiles.append(ht)

            _load_h(0)
            if ngrp > 1:
                _load_h(1)

            cur_stage = None
            p_st = 0
            for gi, (p0, p1) in enumerate(groups):
                if gi + 2 < ngrp:
                    _load_h(gi + 2)
                c0 = int(seg_base[p0])
                ut = tok_tiles[gi]
                ht = h_tiles[gi]
                for p in range(p0, p1):
                    ca, cb = int(seg_base[p]), int(seg_base[p + 1])
                    pq0 = opsum.tile([P, BC // 2], f32, tag="ops")
                    pq1 = opsum.tile([P, BC // 2], f32, tag="ops")
                    for c in range(ca, cb):
                        lc = c - c0
                        hs = ht[:, lc * P : (lc + 1) * P]
                        nc.tensor.matmul(
                            out=pq0[:],
                            lhsT=hs,
                            rhs=ut[:, lc, : BC // 2],
                            start=(c == ca),
                            stop=(c == cb - 1),
                        )
                        nc.tensor.matmul(
                            out=pq1[:],
                            lhsT=hs,
                            rhs=ut[:, lc, BC // 2 :],
                            start=(c == ca),
                            stop=(c == cb - 1),
                        )
                    if cur_stage is None:
                        cur_stage = stagep.tile([P, SGRP, BC], bf16, tag="st")
                        p_st = p
                    nc.scalar.copy(cur_stage[:, p - p_st, : BC // 2], pq0[:])
                    nc.vector.tensor_copy(
                        cur_stage[:, p - p_st, BC // 2 :], pq1[:]
                    )
                    if p - p_st + 1 == SGRP or p == NSEG - 1:
                        nb = p - p_st + 1
                        nc.sync.dma_start(
                            out_d[p_st * P : (p + 1) * P, :].rearrange(
                                "(t r) c -> r t c", t=nb
                            ),
                            cur_stage[:, :nb, :],
                        )
                        cur_stage = None

    nc.finalize()
    return nc


# ---------------------------------------------------------------- entry points

_CACHE = {}


def _get_program(inputs):
    inputs_arr = np.asarray(inputs["inputs"])
    node_ids = np.asarray(inputs["node_ids"])
    clique_ids = np.asarray(inputs["clique_ids"])
    N = int(inputs["nodes"])
    C = int(inputs["n_channels"])
    B, units_dim = inputs_arr.shape
    NC = units_dim // C

    key = (
        B,
        C,
        NC,
        N,
        node_ids.shape[0],
        hash(node_ids.tobytes()),
        hash(clique_ids.tobytes()),
    )
    if key not in _CACHE:
        plan = _plan(node_ids, clique_ids, NC, N)
        nc = _build(plan)
        _CACHE[key] = (plan, nc)
    return _CACHE[key]


def _run(inputs, trace=False):
    inputs_arr = np.asarray(inputs["inputs"]).astype(np.float32)
    N = int(inputs["nodes"])
    C = int(inputs["n_channels"])
    B = inputs_arr.shape[0]
    NC = inputs_arr.shape[1] // C

    plan, nc = _get_program(inputs)
    NR = plan["NR"]
    nseg_r = plan["nseg_r"]

    # clique-major bf16 pooled features, shared by every core
    poolT = np.ascontiguousarray(
        inputs_arr.reshape(B * C, NC).T
    ).astype(ml_dtypes.bfloat16)

    in_maps = []
    for r in range(N_CORES):
        in_maps.append(
            {
                "pooledT": poolT,
                "idxtbl": plan["idx_tbls"][r],
                "htbl": plan["h_tbls"][r],
            }
        )

    res = run_bass_kernel_spmd(
        nc, in_maps, core_ids=list(range(N_CORES)), trace=trace
    )

    out = np.empty((B, C, N), np.float32)
    for r in range(N_CORES):
        pc = plan["per_core"][r]
        starts, widths = pc["starts"], pc["widths"]
        o = np.asarray(res.results[r]["out"]).astype(np.float32)
        blocks = o.reshape(plan["NSEG"], P, B * C)
        full = np.concatenate(
            [blocks[p, : widths[p]] for p in range(nseg_r[r])], axis=0
        )  # [NR, 1024] in node order
        out[:, :, r * NR : (r + 1) * NR] = full.T.reshape(B, C, NR)
    return out.reshape(B, C * N), res


def kernel(**inputs) -> np.ndarray:
    out, _ = _run(inputs, trace=False)
    return out
